# revision 33
# baseline (speedup 1.0000x reference)
"""Trainium2 Bass kernel for nn_AttentionLayer (B=128,H=16,L=64,E=128, C=2048).

out[b,l,:] = (softmax(0.1 * q_bh @ k_bh^T) @ v_bh  for h) . W^T + bias

Strategy: pure data-parallel over batch across 8 NeuronCores (16 batches
per core, no collectives).  Per core:
  - attention per (batch, head-pair) group on the PE using transposed
    ("scores^T") orientation so q/k/v load in natural layout and only
    two [128,128] PE transposes per group are needed,
  - softmax without max-subtraction (scores are bounded: |0.1*s| < ~8),
    rowsum obtained by appending a ones-column to v in the U = exp@v
    matmul, normalization folded into a diag-matmul that also transposes
    U into the V^T layout the output projection needs,
  - output projection  out = V @ W^T + b  as a K=2048 accumulated matmul
    with W pre-transposed on the PE at kernel start.

Matmul dtypes are configurable (bf16 / f32r / f32) for speed vs accuracy.
"""

import numpy as np

import concourse.bass as bass
import concourse.mybir as mybir
import concourse.tile as tile
from concourse import bacc
from concourse.bass_utils import run_bass_kernel_spmd
from concourse.masks import make_identity

N_CORES = 8
B, H, L, E = 128, 16, 64, 128
C = H * E                 # 2048
BPC = B // N_CORES        # 16 batches per core
NBLK = BPC // 2           # 8 two-batch blocks per core
G = H // 2                # 8 head-pair groups per batch
SCALE = 0.1
F32 = mybir.dt.float32
BF16 = mybir.dt.bfloat16

# "bf16" | "f32r" | "f32"
ATT_MODE = "bf16"
MM3_MODE = "bf16"


def _storage_dt(mode):
    return BF16 if mode == "bf16" else F32


def _mm_ap(ap, mode):
    """View an AP with the matmul compute dtype for the given mode."""
    if mode == "f32r":
        return ap.bitcast(mybir.dt.float32r)
    return ap


def emit(ctx, nc, tc, q_d, k_d, v_d, w_d, b_d, o_d, att_mode, mm3_mode,
         skip_wprep=False, skip_attn=False, skip_mm3=False, attn_stage=99):
    att_dt = _storage_dt(att_mode)
    mm3_dt = _storage_dt(mm3_mode)

    # DRAM views: [p, b, g, e] where token row (h*64+l) = g*128 + p
    qv = q_d.rearrange("b h l e -> b (h l) e").rearrange("b (g p) e -> p b g e", p=128)
    kv = k_d.rearrange("b h l e -> b (h l) e").rearrange("b (g p) e -> p b g e", p=128)
    vv = v_d.rearrange("b h l e -> b (h l) e").rearrange("b (g p) e -> p b g e", p=128)

    # f32 weights (f32/f32r projection) eat 128KB/partition — shrink the
    # staging pools to fit
    big_w = mm3_mode != "bf16"
    # f32/f32r attention keeps q/k/v in the f32 staging tiles for the whole
    # block (no bf16 working copies) and computes the softmax row-sum with a
    # separate ones-column matmul instead of a widened v tile.
    att_f32 = att_mode != "bf16"
    const = ctx.enter_context(tc.tile_pool(name="const", bufs=1))
    wst = ctx.enter_context(tc.tile_pool(name="wst", bufs=1 if big_w else 2))
    qkvf = ctx.enter_context(
        tc.tile_pool(name="qkvf", bufs=4 if att_f32 else (2 if big_w else 4))
    )
    if not att_f32:
        qkvb = ctx.enter_context(tc.tile_pool(name="qkvb", bufs=2))
    vtp = ctx.enter_context(tc.tile_pool(name="vtp", bufs=2))
    asml = ctx.enter_context(tc.tile_pool(name="asml", bufs=2 if big_w else 4))
    outp = ctx.enter_context(tc.tile_pool(name="outp", bufs=2))

    # PSUM budget (8 banks): one "tr" bank per group (q/k transposes +
    # V^T regions share a tile) and one "at" bank per group (scores + U),
    # 3 groups in flight each, + 2 banks for the projection.
    pps = ctx.enter_context(tc.tile_pool(name="pps", bufs=3, space="PSUM"))
    pat = ctx.enter_context(tc.tile_pool(name="pat", bufs=3, space="PSUM"))
    pmm3 = ctx.enter_context(tc.tile_pool(name="pmm3", bufs=2, space="PSUM"))

    # ---- constants ----
    identity_att = const.tile([128, 128], att_dt, tag="id_att")
    make_identity(nc, identity_att)
    if mm3_dt is att_dt:
        identity_w = identity_att
    else:
        identity_w = const.tile([128, 128], mm3_dt, tag="id_w")
        make_identity(nc, identity_w)
    # ring of pre-zeroed exp tiles: only the two diagonal 64x64 blocks are
    # ever (re)written, so the off-diagonal blocks stay zero and MM2 can
    # contract over the full 128 partitions without mixing the two heads
    EXPR = 6
    exp_ring = const.tile([128, EXPR, 128], att_dt, tag="expr")
    nc.vector.memset(exp_ring, 0.0)

    bias_bc = const.tile([128, C], F32, tag="bias")
    b_bcast = bass.AP(tensor=b_d.tensor, offset=b_d.offset, ap=[[0, 128]] + list(b_d.ap))
    nc.gpsimd.dma_start(out=bias_bc, in_=b_bcast)

    if att_f32:
        ones_col = const.tile([128, 1], F32, tag="ones")
        nc.vector.memset(ones_col, 1.0)

    # ---- first block's q/k/v loads (emit early so DMA starts early) ----
    def load_block(m):
        qf = qkvf.tile([128, 2, G, 128], F32, tag="qkvf")
        kf = qkvf.tile([128, 2, G, 128], F32, tag="qkvf")
        vf = qkvf.tile([128, 2, G, 128], F32, tag="qkvf")
        nc.sync.dma_start(out=qf, in_=qv[:, 2 * m : 2 * m + 2, :, :])
        nc.sync.dma_start(out=kf, in_=kv[:, 2 * m : 2 * m + 2, :, :])
        nc.sync.dma_start(out=vf, in_=vv[:, 2 * m : 2 * m + 2, :, :])
        if att_f32:
            return qf, kf, vf
        qb = qkvb.tile([128, 2, G, 128], att_dt, tag="qb")
        kb = qkvb.tile([128, 2, G, 128], att_dt, tag="kb")
        nc.gpsimd.tensor_copy(qb, qf)
        nc.gpsimd.tensor_copy(kb, kf)
        vb = qkvb.tile([128, 2, G, 132], att_dt, tag="vb")
        nc.gpsimd.tensor_copy(vb[:, :, :, 0:128], vf)
        nc.gpsimd.memset(vb[:, :, :, 128:129], 1.0)
        return qb, kb, vb

    with nc.named_scope("load0"):
        blk_tiles = load_block(0)

    # ---- W preload + on-chip transpose:  wt_sb[:, kk, n] = W[n, kk*128+p] ----
    wt_sb = const.tile([128, H, C], mm3_dt, tag="wt")
    with nc.named_scope("wprep"):
        for nt in range(16) if not skip_wprep else ():
            wn_f = wst.tile([128, C], F32, tag="wnf")
            nc.sync.dma_start(out=wn_f, in_=w_d[nt * 128 : (nt + 1) * 128, :])
            if mm3_mode == "bf16":
                wn_c = wst.tile([128, C], mm3_dt, tag="wnc")
                nc.gpsimd.tensor_copy(wn_c, wn_f)
            else:
                wn_c = wn_f
            for kk in range(0, 16, 2):
                tp = pps.tile([128, 256], mm3_dt, tag="tr")
                nc.tensor.transpose(
                    tp[:, 0:128], wn_c[:, kk * 128 : (kk + 1) * 128], identity_w
                )
                nc.tensor.transpose(
                    tp[:, 128:256], wn_c[:, (kk + 1) * 128 : (kk + 2) * 128],
                    identity_w,
                )
                nc.any.tensor_copy(
                    wt_sb[:, kk : kk + 2, nt * 128 : (nt + 1) * 128],
                    tp.rearrange("p (a b) -> p a b", a=2),
                )

    # ---- per-block pipeline ----
    for m in range(NBLK):
        qb, kb, vb = blk_tiles
        vt = vtp.tile([128, H, 128], mm3_dt, tag="vt")
        if skip_wprep and m == 0:
            nc.any.memset(wt_sb, 0.01)
        if skip_attn or attn_stage < 7:
            nc.any.memset(vt, 0.01)
        with nc.named_scope(f"attn{m}"):
            for bb in range(2) if not skip_attn else ():
                for g in range(G):
                    # transpose q,k pair-tiles:  [hl,e] -> [e,hl]; one psum
                    # bank carries both transposes + this group's V^T region
                    trp = pps.tile([128, 384], att_dt, tag="tr")
                    qkTp = trp[:, 0:256]
                    nc.tensor.transpose(qkTp[:, 0:128], qb[:, bb, g, :], identity_att)
                    nc.tensor.transpose(qkTp[:, 128:256], kb[:, bb, g, :], identity_att)
                    qkT = asml.tile([128, 256], att_dt, tag="qkT")
                    nc.vector.tensor_copy(qkT, qkTp)
                    qT2 = qkT[:, 0:128]
                    kT2 = qkT[:, 128:256]

                    if attn_stage < 2:
                        continue
                    # One psum bank holds this group's scores^T (cols 0:128)
                    # and U' = exp@[v|1] (cols 128:257).
                    # Full 128x128 scores^T: diagonal 64x64 blocks are the two
                    # heads' k^T q; off-diagonal blocks are cross-head garbage
                    # we never read.  (Full-row matmuls only — 64-row PE tile
                    # configs fault TRN2 when consecutive matmuls move between
                    # row positions.)
                    at = pat.tile([128, 260], F32, tag="at")
                    scT = at[:, 0:128]
                    nc.tensor.matmul(
                        scT, _mm_ap(kT2, att_mode), _mm_ap(qT2, att_mode),
                        start=True, stop=True,
                    )

                    if attn_stage < 3:
                        continue
                    # exp(scale * scores^T) into a pre-zeroed ring slot: only
                    # the diagonal blocks are written, so expT is block-diagonal
                    # and MM2 can contract over all 128 partitions.
                    expT = exp_ring[:, (bb * G + g) % EXPR, :]
                    for lo, hi in ((0, 64), (64, 128)):
                        nc.scalar.activation(
                            expT[lo:hi, lo:hi], scT[lo:hi, lo:hi],
                            mybir.ActivationFunctionType.Exp, scale=SCALE,
                        )

                    if attn_stage < 4:
                        continue
                    # U = exp @ [v | 1]  -> token-major U plus rowsum column
                    U2p = at[:, 128:257]
                    if att_f32:
                        nc.tensor.matmul(
                            U2p[:, 0:128],
                            _mm_ap(expT, att_mode),
                            _mm_ap(vb[:, bb, g, 0:128], att_mode),
                            start=True, stop=True,
                        )
                        nc.tensor.matmul(
                            U2p[:, 128:129],
                            _mm_ap(expT, att_mode),
                            _mm_ap(ones_col, att_mode),
                            start=True, stop=True,
                        )
                    else:
                        nc.tensor.matmul(
                            U2p,
                            _mm_ap(expT, att_mode),
                            _mm_ap(vb[:, bb, g, 0:129], att_mode),
                            start=True, stop=True,
                        )

                    if attn_stage < 5:
                        continue
                    # normalize in token-major form: V[l2,d] = U[l2,d]/rowsum[l2]
                    # (per-partition scalar — the natural broadcast direction)
                    r2 = asml.tile([128, 1], F32, tag="r2")
                    nc.vector.reciprocal(r2, U2p[:, 128:129])
                    V2 = asml.tile([128, 128], att_dt, tag="V2")
                    nc.vector.tensor_scalar_mul(V2, U2p[:, 0:128], r2)

                    if attn_stage < 6:
                        continue
                    # transpose V into the c-major layout MM3's stationary needs
                    VT2p = trp[:, 256:384]
                    nc.tensor.transpose(VT2p, V2, identity_att)
                    if attn_stage < 7:
                        continue
                    tok = bb * 64
                    nc.any.tensor_copy(
                        vt[:, 2 * g : 2 * g + 2, tok : tok + 64],
                        VT2p.rearrange("p (a b) -> p a b", a=2),
                    )

        # prefetch next block while this block's projection runs
        if m + 1 < NBLK:
            with nc.named_scope(f"load{m + 1}"):
                blk_tiles = load_block(m + 1)

        # ---- output projection for this block's 128 tokens ----
        if skip_mm3:
            ot = outp.tile([128, C], F32, tag="ot_dbg")
            nc.any.tensor_copy(ot, vt.rearrange("p h t -> p (h t)"))
            nc.sync.dma_start(out=o_d[m * 128 : (m + 1) * 128, :], in_=ot)
            continue
        with nc.named_scope(f"proj{m}"):
            ot = outp.tile([128, C], F32, tag="ot")
            for half in range(2):
                ps = [
                    pmm3.tile([128, 512], F32, tag="mm3", name=f"ps{n}")
                    for n in range(2)
                ]
                for kk in range(16):
                    for n in range(2):
                        nn = half * 2 + n
                        nc.tensor.matmul(
                            ps[n],
                            _mm_ap(vt[:, kk, :], mm3_mode),
                            _mm_ap(wt_sb[:, kk, nn * 512 : (nn + 1) * 512], mm3_mode),
                            start=(kk == 0), stop=(kk == 15),
                        )
                for n in range(2):
                    nn = half * 2 + n
                    nc.any.tensor_add(
                        ot[:, nn * 512 : (nn + 1) * 512],
                        ps[n],
                        bias_bc[:, nn * 512 : (nn + 1) * 512],
                    )
            nc.sync.dma_start(out=o_d[m * 128 : (m + 1) * 128, :], in_=ot)


def build(att_mode=ATT_MODE, mm3_mode=MM3_MODE, **emit_kwargs):
    import contextlib

    nc = bacc.Bacc("TRN2", target_bir_lowering=False, debug=False)
    q_d = nc.dram_tensor("queries", [BPC, H, L, E], F32, kind="ExternalInput").ap()
    k_d = nc.dram_tensor("keys", [BPC, H, L, E], F32, kind="ExternalInput").ap()
    v_d = nc.dram_tensor("values", [BPC, H, L, E], F32, kind="ExternalInput").ap()
    w_d = nc.dram_tensor("W", [C, C], F32, kind="ExternalInput").ap()
    b_d = nc.dram_tensor("b", [C], F32, kind="ExternalInput").ap()
    o_d = nc.dram_tensor("out", [BPC * L, C], F32, kind="ExternalOutput").ap()

    with tile.TileContext(nc) as tc:
        with contextlib.ExitStack() as ctx:
            emit(ctx, nc, tc, q_d, k_d, v_d, w_d, b_d, o_d, att_mode, mm3_mode,
                 **emit_kwargs)
    nc.compile()
    return nc


_NC_CACHE = {}


def get_nc(att_mode=ATT_MODE, mm3_mode=MM3_MODE):
    key = (att_mode, mm3_mode)
    if key not in _NC_CACHE:
        _NC_CACHE[key] = build(att_mode, mm3_mode)
    return _NC_CACHE[key]


def make_in_maps(queries, keys, values, W, b):
    queries = np.ascontiguousarray(np.asarray(queries, dtype=np.float32))
    keys = np.ascontiguousarray(np.asarray(keys, dtype=np.float32))
    values = np.ascontiguousarray(np.asarray(values, dtype=np.float32))
    W = np.ascontiguousarray(np.asarray(W, dtype=np.float32))
    b = np.ascontiguousarray(np.asarray(b, dtype=np.float32))
    in_maps = []
    for i in range(N_CORES):
        s = slice(i * BPC, (i + 1) * BPC)
        in_maps.append(
            {
                "queries": queries[s],
                "keys": keys[s],
                "values": values[s],
                "W": W,
                "b": b,
            }
        )
    return in_maps


def kernel(queries, keys, values, W, b, **run_kwargs):
    nc = get_nc()
    in_maps = make_in_maps(queries, keys, values, W, b)
    res = run_bass_kernel_spmd(nc, in_maps, core_ids=list(range(N_CORES)), **run_kwargs)
    out = np.concatenate([res.results[i]["out"] for i in range(N_CORES)], axis=0)
    return out.reshape(B, L, C)


# revision 35
# speedup vs baseline: 1.2357x; 1.2357x over previous
"""Trainium2 Bass kernel for nn_AttentionLayer (B=128,H=16,L=64,E=128, C=2048).

out[b,l,:] = (softmax(0.1 * q_bh @ k_bh^T) @ v_bh  for h) . W^T + bias

Strategy: pure data-parallel over batch across 8 NeuronCores (16 batches
per core, no collectives).  Per core:
  - attention per (batch, head-pair) group on the PE using transposed
    ("scores^T") orientation so q/k/v load in natural layout and only
    two [128,128] PE transposes per group are needed,
  - softmax without max-subtraction (scores are bounded: |0.1*s| < ~8),
    rowsum obtained by appending a ones-column to v in the U = exp@v
    matmul, normalization folded into a diag-matmul that also transposes
    U into the V^T layout the output projection needs,
  - output projection  out = V @ W^T + b  as a K=2048 accumulated matmul
    with W pre-transposed on the PE at kernel start.

Matmul dtypes are configurable (bf16 / f32r / f32) for speed vs accuracy.
"""

import numpy as np

import concourse.bass as bass
import concourse.mybir as mybir
import concourse.tile as tile
from concourse import bacc
from concourse.bass_utils import run_bass_kernel_spmd
from concourse.masks import make_identity

N_CORES = 8
B, H, L, E = 128, 16, 64, 128
C = H * E                 # 2048
BPC = B // N_CORES        # 16 batches per core
NBLK = BPC // 2           # 8 two-batch blocks per core
G = H // 2                # 8 head-pair groups per batch
SCALE = 0.1
F32 = mybir.dt.float32
BF16 = mybir.dt.bfloat16

# "bf16" | "f32r" | "f32"
ATT_MODE = "bf16"
MM3_MODE = "bf16"


def _storage_dt(mode):
    return BF16 if mode == "bf16" else F32


def _mm_ap(ap, mode):
    """View an AP with the matmul compute dtype for the given mode."""
    if mode == "f32r":
        return ap.bitcast(mybir.dt.float32r)
    return ap


def emit(ctx, nc, tc, q_d, k_d, v_d, w_d, b_d, o_d, att_mode, mm3_mode,
         skip_wprep=False, skip_attn=False, skip_mm3=False, attn_stage=99):
    att_dt = _storage_dt(att_mode)
    mm3_dt = _storage_dt(mm3_mode)

    # DRAM views: [p, b, g, e] where token row (h*64+l) = g*128 + p
    qv = q_d.rearrange("b h l e -> b (h l) e").rearrange("b (g p) e -> p b g e", p=128)
    kv = k_d.rearrange("b h l e -> b (h l) e").rearrange("b (g p) e -> p b g e", p=128)
    vv = v_d.rearrange("b h l e -> b (h l) e").rearrange("b (g p) e -> p b g e", p=128)

    # f32 weights (f32/f32r projection) eat 128KB/partition — shrink the
    # staging pools to fit
    big_w = mm3_mode != "bf16"
    # f32/f32r attention keeps q/k/v in the f32 staging tiles for the whole
    # block (no bf16 working copies) and computes the softmax row-sum with a
    # separate ones-column matmul instead of a widened v tile.
    att_f32 = att_mode != "bf16"
    const = ctx.enter_context(tc.tile_pool(name="const", bufs=1))
    wst = ctx.enter_context(tc.tile_pool(name="wst", bufs=1 if big_w else 2))
    qkvf = ctx.enter_context(
        tc.tile_pool(name="qkvf", bufs=4 if att_f32 else (2 if big_w else 4))
    )
    if not att_f32:
        qkvb = ctx.enter_context(tc.tile_pool(name="qkvb", bufs=2))
    vtp = ctx.enter_context(tc.tile_pool(name="vtp", bufs=2))
    qktp = ctx.enter_context(tc.tile_pool(name="qktp", bufs=2))
    asml = ctx.enter_context(tc.tile_pool(name="asml", bufs=2 if big_w else 4))
    outp = ctx.enter_context(tc.tile_pool(name="outp", bufs=2))

    # PSUM budget (8 banks): "tr" 3 (q/k batch transposes + V^T tiles),
    # "at" 3 (scores + U, one bank per group), + 2 banks for the projection.
    pps = ctx.enter_context(tc.tile_pool(name="pps", bufs=3, space="PSUM"))
    pat = ctx.enter_context(tc.tile_pool(name="pat", bufs=3, space="PSUM"))
    pmm3 = ctx.enter_context(tc.tile_pool(name="pmm3", bufs=2, space="PSUM"))

    # ---- constants ----
    identity_att = const.tile([128, 128], att_dt, tag="id_att")
    make_identity(nc, identity_att)
    if mm3_dt is att_dt:
        identity_w = identity_att
    else:
        identity_w = const.tile([128, 128], mm3_dt, tag="id_w")
        make_identity(nc, identity_w)
    # ring of pre-zeroed exp tiles: only the two diagonal 64x64 blocks are
    # ever (re)written, so the off-diagonal blocks stay zero and MM2 can
    # contract over the full 128 partitions without mixing the two heads
    EXPR = 6
    exp_ring = const.tile([128, EXPR, 128], att_dt, tag="expr")
    nc.vector.memset(exp_ring, 0.0)

    bias_bc = const.tile([128, C], F32, tag="bias")
    b_bcast = bass.AP(tensor=b_d.tensor, offset=b_d.offset, ap=[[0, 128]] + list(b_d.ap))
    nc.gpsimd.dma_start(out=bias_bc, in_=b_bcast)

    if att_f32:
        ones_col = const.tile([128, 1], F32, tag="ones")
        nc.vector.memset(ones_col, 1.0)

    # ---- first block's q/k/v loads (emit early so DMA starts early) ----
    def load_block(m):
        qf = qkvf.tile([128, 2, G, 128], F32, tag="qkvf")
        kf = qkvf.tile([128, 2, G, 128], F32, tag="qkvf")
        vf = qkvf.tile([128, 2, G, 128], F32, tag="qkvf")
        nc.sync.dma_start(out=qf, in_=qv[:, 2 * m : 2 * m + 2, :, :])
        nc.sync.dma_start(out=kf, in_=kv[:, 2 * m : 2 * m + 2, :, :])
        nc.sync.dma_start(out=vf, in_=vv[:, 2 * m : 2 * m + 2, :, :])
        if att_f32:
            return qf, kf, vf
        qb = qkvb.tile([128, 2, G, 128], att_dt, tag="qb")
        kb = qkvb.tile([128, 2, G, 128], att_dt, tag="kb")
        nc.any.tensor_copy(qb, qf)
        nc.any.tensor_copy(kb, kf)
        vb = qkvb.tile([128, 2, G, 132], att_dt, tag="vb")
        nc.any.tensor_copy(vb[:, :, :, 0:128], vf)
        nc.vector.memset(vb[:, :, :, 128:129], 1.0)
        return qb, kb, vb

    with nc.named_scope("load0"):
        blk_tiles = load_block(0)

    # ---- W preload + on-chip transpose:  wt_sb[:, kk, n] = W[n, kk*128+p] ----
    wt_sb = const.tile([128, H, C], mm3_dt, tag="wt")
    with nc.named_scope("wprep"):
        for nt in range(16) if not skip_wprep else ():
            wn_f = wst.tile([128, C], F32, tag="wnf")
            nc.sync.dma_start(out=wn_f, in_=w_d[nt * 128 : (nt + 1) * 128, :])
            if mm3_mode == "bf16":
                wn_c = wst.tile([128, C], mm3_dt, tag="wnc")
                nc.any.tensor_copy(wn_c, wn_f)
            else:
                wn_c = wn_f
            for kk in range(0, 16, 2):
                tp = pps.tile([128, 256], mm3_dt, tag="tr")
                nc.tensor.transpose(
                    tp[:, 0:128], wn_c[:, kk * 128 : (kk + 1) * 128], identity_w
                )
                nc.tensor.transpose(
                    tp[:, 128:256], wn_c[:, (kk + 1) * 128 : (kk + 2) * 128],
                    identity_w,
                )
                nc.any.tensor_copy(
                    wt_sb[:, kk : kk + 2, nt * 128 : (nt + 1) * 128],
                    tp.rearrange("p (a b) -> p a b", a=2),
                )

    # ---- per-block pipeline ----
    for m in range(NBLK):
        qb, kb, vb = blk_tiles
        vt = vtp.tile([128, H, 128], mm3_dt, tag="vt")
        if skip_wprep and m == 0:
            nc.any.memset(wt_sb, 0.01)
        if skip_attn or attn_stage < 7:
            nc.any.memset(vt, 0.01)
        with nc.named_scope(f"attn{m}"):
            # batch-transpose this block's q and k up front — keeps the
            # per-group chain short and lets transposes run ahead during the
            # previous block's projection
            qkt = qktp.tile([128, 2, G, 256], att_dt, tag="qkt")
            if not skip_attn:
                for bb in range(2):
                    for g in range(G):
                        trp = pps.tile([128, 256], att_dt, tag="tr")
                        nc.tensor.transpose(trp[:, 0:128], qb[:, bb, g, :], identity_att)
                        nc.tensor.transpose(trp[:, 128:256], kb[:, bb, g, :], identity_att)
                        nc.vector.tensor_copy(qkt[:, bb, g, :], trp)
            for bb in range(2) if not skip_attn else ():
                for g in range(G):
                    qT2 = qkt[:, bb, g, 0:128]
                    kT2 = qkt[:, bb, g, 128:256]

                    if attn_stage < 2:
                        continue
                    # One psum bank holds this group's scores^T (cols 0:128)
                    # and U' = exp@[v|1] (cols 128:257).
                    # Full 128x128 scores^T: diagonal 64x64 blocks are the two
                    # heads' k^T q; off-diagonal blocks are cross-head garbage
                    # we never read.  (Full-row matmuls only — 64-row PE tile
                    # configs fault TRN2 when consecutive matmuls move between
                    # row positions.)
                    at = pat.tile([128, 260], F32, tag="at")
                    scT = at[:, 0:128]
                    nc.tensor.matmul(
                        scT, _mm_ap(kT2, att_mode), _mm_ap(qT2, att_mode),
                        start=True, stop=True,
                    )

                    if attn_stage < 3:
                        continue
                    # exp(scale * scores^T) into a pre-zeroed ring slot: only
                    # the diagonal blocks are written, so expT is block-diagonal
                    # and MM2 can contract over all 128 partitions.
                    expT = exp_ring[:, (bb * G + g) % EXPR, :]
                    for lo, hi in ((0, 64), (64, 128)):
                        nc.scalar.activation(
                            expT[lo:hi, lo:hi], scT[lo:hi, lo:hi],
                            mybir.ActivationFunctionType.Exp, scale=SCALE,
                        )

                    if attn_stage < 4:
                        continue
                    # U = exp @ [v | 1]  -> token-major U plus rowsum column
                    U2p = at[:, 128:257]
                    if att_f32:
                        nc.tensor.matmul(
                            U2p[:, 0:128],
                            _mm_ap(expT, att_mode),
                            _mm_ap(vb[:, bb, g, 0:128], att_mode),
                            start=True, stop=True,
                        )
                        nc.tensor.matmul(
                            U2p[:, 128:129],
                            _mm_ap(expT, att_mode),
                            _mm_ap(ones_col, att_mode),
                            start=True, stop=True,
                        )
                    else:
                        nc.tensor.matmul(
                            U2p,
                            _mm_ap(expT, att_mode),
                            _mm_ap(vb[:, bb, g, 0:129], att_mode),
                            start=True, stop=True,
                        )

                    if attn_stage < 5:
                        continue
                    # normalize in token-major form: V[l2,d] = U[l2,d]/rowsum[l2]
                    # (per-partition scalar — the natural broadcast direction)
                    r2 = asml.tile([128, 1], F32, tag="r2")
                    nc.vector.reciprocal(r2, U2p[:, 128:129])
                    V2 = asml.tile([128, 128], att_dt, tag="V2")
                    nc.vector.tensor_scalar_mul(V2, U2p[:, 0:128], r2)

                    if attn_stage < 6:
                        continue
                    # transpose V into the c-major layout MM3's stationary needs
                    VT2p = pps.tile([128, 128], att_dt, tag="tr")
                    nc.tensor.transpose(VT2p, V2, identity_att)
                    if attn_stage < 7:
                        continue
                    tok = bb * 64
                    nc.any.tensor_copy(
                        vt[:, 2 * g : 2 * g + 2, tok : tok + 64],
                        VT2p.rearrange("p (a b) -> p a b", a=2),
                    )

        # prefetch next block while this block's projection runs
        if m + 1 < NBLK:
            with nc.named_scope(f"load{m + 1}"):
                blk_tiles = load_block(m + 1)

        # ---- output projection for this block's 128 tokens ----
        if skip_mm3:
            ot = outp.tile([128, C], F32, tag="ot_dbg")
            nc.any.tensor_copy(ot, vt.rearrange("p h t -> p (h t)"))
            nc.sync.dma_start(out=o_d[m * 128 : (m + 1) * 128, :], in_=ot)
            continue
        with nc.named_scope(f"proj{m}"):
            ot = outp.tile([128, C], F32, tag="ot")
            for half in range(2):
                ps = [
                    pmm3.tile([128, 512], F32, tag="mm3", name=f"ps{n}")
                    for n in range(2)
                ]
                for kk in range(16):
                    for n in range(2):
                        nn = half * 2 + n
                        nc.tensor.matmul(
                            ps[n],
                            _mm_ap(vt[:, kk, :], mm3_mode),
                            _mm_ap(wt_sb[:, kk, nn * 512 : (nn + 1) * 512], mm3_mode),
                            start=(kk == 0), stop=(kk == 15),
                        )
                for n in range(2):
                    nn = half * 2 + n
                    nc.any.tensor_add(
                        ot[:, nn * 512 : (nn + 1) * 512],
                        ps[n],
                        bias_bc[:, nn * 512 : (nn + 1) * 512],
                    )
            nc.sync.dma_start(out=o_d[m * 128 : (m + 1) * 128, :], in_=ot)


def build(att_mode=ATT_MODE, mm3_mode=MM3_MODE, **emit_kwargs):
    import contextlib

    nc = bacc.Bacc("TRN2", target_bir_lowering=False, debug=False)
    q_d = nc.dram_tensor("queries", [BPC, H, L, E], F32, kind="ExternalInput").ap()
    k_d = nc.dram_tensor("keys", [BPC, H, L, E], F32, kind="ExternalInput").ap()
    v_d = nc.dram_tensor("values", [BPC, H, L, E], F32, kind="ExternalInput").ap()
    w_d = nc.dram_tensor("W", [C, C], F32, kind="ExternalInput").ap()
    b_d = nc.dram_tensor("b", [C], F32, kind="ExternalInput").ap()
    o_d = nc.dram_tensor("out", [BPC * L, C], F32, kind="ExternalOutput").ap()

    with tile.TileContext(nc) as tc:
        with contextlib.ExitStack() as ctx:
            emit(ctx, nc, tc, q_d, k_d, v_d, w_d, b_d, o_d, att_mode, mm3_mode,
                 **emit_kwargs)
    nc.compile()
    return nc


_NC_CACHE = {}


def get_nc(att_mode=ATT_MODE, mm3_mode=MM3_MODE):
    key = (att_mode, mm3_mode)
    if key not in _NC_CACHE:
        _NC_CACHE[key] = build(att_mode, mm3_mode)
    return _NC_CACHE[key]


def make_in_maps(queries, keys, values, W, b):
    queries = np.ascontiguousarray(np.asarray(queries, dtype=np.float32))
    keys = np.ascontiguousarray(np.asarray(keys, dtype=np.float32))
    values = np.ascontiguousarray(np.asarray(values, dtype=np.float32))
    W = np.ascontiguousarray(np.asarray(W, dtype=np.float32))
    b = np.ascontiguousarray(np.asarray(b, dtype=np.float32))
    in_maps = []
    for i in range(N_CORES):
        s = slice(i * BPC, (i + 1) * BPC)
        in_maps.append(
            {
                "queries": queries[s],
                "keys": keys[s],
                "values": values[s],
                "W": W,
                "b": b,
            }
        )
    return in_maps


def kernel(queries, keys, values, W, b, **run_kwargs):
    nc = get_nc()
    in_maps = make_in_maps(queries, keys, values, W, b)
    res = run_bass_kernel_spmd(nc, in_maps, core_ids=list(range(N_CORES)), **run_kwargs)
    out = np.concatenate([res.results[i]["out"] for i in range(N_CORES)], axis=0)
    return out.reshape(B, L, C)


# revision 36
# speedup vs baseline: 1.2546x; 1.0152x over previous
"""Trainium2 Bass kernel for nn_AttentionLayer (B=128,H=16,L=64,E=128, C=2048).

out[b,l,:] = (softmax(0.1 * q_bh @ k_bh^T) @ v_bh  for h) . W^T + bias

Strategy: pure data-parallel over batch across 8 NeuronCores (16 batches
per core, no collectives).  Per core:
  - attention per (batch, head-pair) group on the PE using transposed
    ("scores^T") orientation so q/k/v load in natural layout and only
    two [128,128] PE transposes per group are needed,
  - softmax without max-subtraction (scores are bounded: |0.1*s| < ~8),
    rowsum obtained by appending a ones-column to v in the U = exp@v
    matmul, normalization folded into a diag-matmul that also transposes
    U into the V^T layout the output projection needs,
  - output projection  out = V @ W^T + b  as a K=2048 accumulated matmul
    with W pre-transposed on the PE at kernel start.

Matmul dtypes are configurable (bf16 / f32r / f32) for speed vs accuracy.
"""

import numpy as np

import concourse.bass as bass
import concourse.mybir as mybir
import concourse.tile as tile
from concourse import bacc
from concourse.bass_utils import run_bass_kernel_spmd
from concourse.masks import make_identity

N_CORES = 8
B, H, L, E = 128, 16, 64, 128
C = H * E                 # 2048
BPC = B // N_CORES        # 16 batches per core
NBLK = BPC // 2           # 8 two-batch blocks per core
G = H // 2                # 8 head-pair groups per batch
SCALE = 0.1
F32 = mybir.dt.float32
BF16 = mybir.dt.bfloat16

# "bf16" | "f32r" | "f32"
ATT_MODE = "bf16"
MM3_MODE = "bf16"


def _storage_dt(mode):
    return BF16 if mode == "bf16" else F32


def _mm_ap(ap, mode):
    """View an AP with the matmul compute dtype for the given mode."""
    if mode == "f32r":
        return ap.bitcast(mybir.dt.float32r)
    return ap


def emit(ctx, nc, tc, q_d, k_d, v_d, w_d, b_d, o_d, att_mode, mm3_mode,
         skip_wprep=False, skip_attn=False, skip_mm3=False, attn_stage=99):
    att_dt = _storage_dt(att_mode)
    mm3_dt = _storage_dt(mm3_mode)

    # DRAM views: [p, b, g, e] where token row (h*64+l) = g*128 + p
    qv = q_d.rearrange("b h l e -> b (h l) e").rearrange("b (g p) e -> p b g e", p=128)
    kv = k_d.rearrange("b h l e -> b (h l) e").rearrange("b (g p) e -> p b g e", p=128)
    vv = v_d.rearrange("b h l e -> b (h l) e").rearrange("b (g p) e -> p b g e", p=128)

    # f32 weights (f32/f32r projection) eat 128KB/partition — shrink the
    # staging pools to fit
    big_w = mm3_mode != "bf16"
    # f32/f32r attention keeps q/k/v in the f32 staging tiles for the whole
    # block (no bf16 working copies) and computes the softmax row-sum with a
    # separate ones-column matmul instead of a widened v tile.
    att_f32 = att_mode != "bf16"
    const = ctx.enter_context(tc.tile_pool(name="const", bufs=1))
    wst = ctx.enter_context(tc.tile_pool(name="wst", bufs=1 if big_w else 2))
    qkvf = ctx.enter_context(
        tc.tile_pool(name="qkvf", bufs=4 if att_f32 else (2 if big_w else 4))
    )
    if not att_f32:
        qkvb = ctx.enter_context(tc.tile_pool(name="qkvb", bufs=2))
    vtp = ctx.enter_context(tc.tile_pool(name="vtp", bufs=2))
    qktp = ctx.enter_context(tc.tile_pool(name="qktp", bufs=2))
    asml = ctx.enter_context(tc.tile_pool(name="asml", bufs=2 if big_w else 4))
    outp = ctx.enter_context(tc.tile_pool(name="outp", bufs=2))

    # PSUM budget (8 banks): "tr" 3 (q/k batch transposes + V^T tiles),
    # "at" 3 (scores + U, one bank per group), + 2 banks for the projection.
    pps = ctx.enter_context(tc.tile_pool(name="pps", bufs=2, space="PSUM"))
    pat = ctx.enter_context(tc.tile_pool(name="pat", bufs=4, space="PSUM"))
    pmm3 = ctx.enter_context(tc.tile_pool(name="pmm3", bufs=2, space="PSUM"))

    # ---- constants ----
    identity_att = const.tile([128, 128], att_dt, tag="id_att")
    make_identity(nc, identity_att)
    if att_dt is F32:
        identity_f32 = identity_att
    else:
        identity_f32 = const.tile([128, 128], F32, tag="id_f32")
        make_identity(nc, identity_f32)
    if mm3_dt is att_dt:
        identity_w = identity_att
    else:
        identity_w = const.tile([128, 128], mm3_dt, tag="id_w")
        make_identity(nc, identity_w)
    # ring of pre-zeroed exp tiles: only the two diagonal 64x64 blocks are
    # ever (re)written, so the off-diagonal blocks stay zero and MM2 can
    # contract over the full 128 partitions without mixing the two heads
    EXPR = 6
    exp_ring = const.tile([128, EXPR, 128], att_dt, tag="expr")
    nc.vector.memset(exp_ring, 0.0)

    bias_bc = const.tile([128, C], F32, tag="bias")
    b_bcast = bass.AP(tensor=b_d.tensor, offset=b_d.offset, ap=[[0, 128]] + list(b_d.ap))
    nc.gpsimd.dma_start(out=bias_bc, in_=b_bcast)

    if att_f32:
        ones_col = const.tile([128, 1], F32, tag="ones")
        nc.vector.memset(ones_col, 1.0)

    # ---- first block's q/k/v loads (emit early so DMA starts early) ----
    def load_block(m):
        qf = qkvf.tile([128, 2, G, 128], F32, tag="qkvf")
        kf = qkvf.tile([128, 2, G, 128], F32, tag="qkvf")
        vf = qkvf.tile([128, 2, G, 128], F32, tag="qkvf")
        nc.sync.dma_start(out=qf, in_=qv[:, 2 * m : 2 * m + 2, :, :])
        nc.sync.dma_start(out=kf, in_=kv[:, 2 * m : 2 * m + 2, :, :])
        nc.sync.dma_start(out=vf, in_=vv[:, 2 * m : 2 * m + 2, :, :])
        if att_f32:
            return qf, kf, vf
        qb = qkvb.tile([128, 2, G, 128], att_dt, tag="qb")
        kb = qkvb.tile([128, 2, G, 128], att_dt, tag="kb")
        nc.any.tensor_copy(qb, qf)
        nc.any.tensor_copy(kb, kf)
        vb = qkvb.tile([128, 2, G, 132], att_dt, tag="vb")
        nc.any.tensor_copy(vb[:, :, :, 0:128], vf)
        nc.vector.memset(vb[:, :, :, 128:129], 1.0)
        return qb, kb, vb

    with nc.named_scope("load0"):
        blk_tiles = load_block(0)

    # ---- W preload + on-chip transpose:  wt_sb[:, kk, n] = W[n, kk*128+p] ----
    wt_sb = const.tile([128, H, C], mm3_dt, tag="wt")
    with nc.named_scope("wprep"):
        for nt in range(16) if not skip_wprep else ():
            wn_f = wst.tile([128, C], F32, tag="wnf")
            nc.sync.dma_start(out=wn_f, in_=w_d[nt * 128 : (nt + 1) * 128, :])
            if mm3_mode == "bf16":
                wn_c = wst.tile([128, C], mm3_dt, tag="wnc")
                nc.any.tensor_copy(wn_c, wn_f)
            else:
                wn_c = wn_f
            for kk in range(0, 16, 2):
                tp = pps.tile([128, 256], mm3_dt, tag="tr")
                nc.tensor.transpose(
                    tp[:, 0:128], wn_c[:, kk * 128 : (kk + 1) * 128], identity_w
                )
                nc.tensor.transpose(
                    tp[:, 128:256], wn_c[:, (kk + 1) * 128 : (kk + 2) * 128],
                    identity_w,
                )
                nc.any.tensor_copy(
                    wt_sb[:, kk : kk + 2, nt * 128 : (nt + 1) * 128],
                    tp.rearrange("p (a b) -> p a b", a=2),
                )

    # ---- per-block pipeline ----
    for m in range(NBLK):
        qb, kb, vb = blk_tiles
        vt = vtp.tile([128, H, 128], mm3_dt, tag="vt")
        if skip_wprep and m == 0:
            nc.any.memset(wt_sb, 0.01)
        if skip_attn or attn_stage < 7:
            nc.any.memset(vt, 0.01)
        with nc.named_scope(f"attn{m}"):
            # batch-transpose this block's q and k up front — keeps the
            # per-group chain short and lets transposes run ahead during the
            # previous block's projection
            qkt = qktp.tile([128, 2, G, 256], att_dt, tag="qkt")
            if not skip_attn:
                for bb in range(2):
                    for g in range(G):
                        trp = pps.tile([128, 256], att_dt, tag="tr")
                        nc.tensor.transpose(trp[:, 0:128], qb[:, bb, g, :], identity_att)
                        nc.tensor.transpose(trp[:, 128:256], kb[:, bb, g, :], identity_att)
                        nc.vector.tensor_copy(qkt[:, bb, g, :], trp)
            for bb in range(2) if not skip_attn else ():
                for g in range(G):
                    qT2 = qkt[:, bb, g, 0:128]
                    kT2 = qkt[:, bb, g, 128:256]

                    if attn_stage < 2:
                        continue
                    # One psum bank holds this group's scores^T (cols 0:128)
                    # and U' = exp@[v|1] (cols 128:257).
                    # Full 128x128 scores^T: diagonal 64x64 blocks are the two
                    # heads' k^T q; off-diagonal blocks are cross-head garbage
                    # we never read.  (Full-row matmuls only — 64-row PE tile
                    # configs fault TRN2 when consecutive matmuls move between
                    # row positions.)
                    at = pat.tile([128, 392], F32, tag="at")
                    scT = at[:, 0:128]
                    nc.tensor.matmul(
                        scT, _mm_ap(kT2, att_mode), _mm_ap(qT2, att_mode),
                        start=True, stop=True,
                    )

                    if attn_stage < 3:
                        continue
                    # exp(scale * scores^T) into a pre-zeroed ring slot: only
                    # the diagonal blocks are written, so expT is block-diagonal
                    # and MM2 can contract over all 128 partitions.
                    expT = exp_ring[:, (bb * G + g) % EXPR, :]
                    for lo, hi in ((0, 64), (64, 128)):
                        nc.scalar.activation(
                            expT[lo:hi, lo:hi], scT[lo:hi, lo:hi],
                            mybir.ActivationFunctionType.Exp, scale=SCALE,
                        )

                    if attn_stage < 4:
                        continue
                    # U = exp @ [v | 1]  -> token-major U plus rowsum column
                    U2p = at[:, 128:257]
                    if att_f32:
                        nc.tensor.matmul(
                            U2p[:, 0:128],
                            _mm_ap(expT, att_mode),
                            _mm_ap(vb[:, bb, g, 0:128], att_mode),
                            start=True, stop=True,
                        )
                        nc.tensor.matmul(
                            U2p[:, 128:129],
                            _mm_ap(expT, att_mode),
                            _mm_ap(ones_col, att_mode),
                            start=True, stop=True,
                        )
                    else:
                        nc.tensor.matmul(
                            U2p,
                            _mm_ap(expT, att_mode),
                            _mm_ap(vb[:, bb, g, 0:129], att_mode),
                            start=True, stop=True,
                        )

                    if attn_stage < 5:
                        continue
                    # normalize in token-major form: V[l2,d] = U[l2,d]/rowsum[l2]
                    # (per-partition scalar — the natural broadcast direction)
                    r2 = asml.tile([128, 1], F32, tag="r2")
                    nc.vector.reciprocal(r2, U2p[:, 128:129])
                    V2 = asml.tile([128, 128], F32, tag="V2")
                    nc.vector.tensor_scalar_mul(V2, U2p[:, 0:128], r2)

                    if attn_stage < 6:
                        continue
                    # transpose V into the c-major layout MM3's stationary needs
                    # (f32, into the spare region of this group's psum bank)
                    VT2p = at[:, 260:388]
                    nc.tensor.transpose(VT2p, V2, identity_f32)
                    if attn_stage < 7:
                        continue
                    tok = bb * 64
                    nc.vector.tensor_copy(
                        vt[:, 2 * g : 2 * g + 2, tok : tok + 64],
                        VT2p.rearrange("p (a b) -> p a b", a=2),
                    )

        # prefetch next block while this block's projection runs
        if m + 1 < NBLK:
            with nc.named_scope(f"load{m + 1}"):
                blk_tiles = load_block(m + 1)

        # ---- output projection for this block's 128 tokens ----
        if skip_mm3:
            ot = outp.tile([128, C], F32, tag="ot_dbg")
            nc.any.tensor_copy(ot, vt.rearrange("p h t -> p (h t)"))
            nc.sync.dma_start(out=o_d[m * 128 : (m + 1) * 128, :], in_=ot)
            continue
        with nc.named_scope(f"proj{m}"):
            ot = outp.tile([128, C], F32, tag="ot")
            for half in range(2):
                ps = [
                    pmm3.tile([128, 512], F32, tag="mm3", name=f"ps{n}")
                    for n in range(2)
                ]
                for kk in range(16):
                    for n in range(2):
                        nn = half * 2 + n
                        nc.tensor.matmul(
                            ps[n],
                            _mm_ap(vt[:, kk, :], mm3_mode),
                            _mm_ap(wt_sb[:, kk, nn * 512 : (nn + 1) * 512], mm3_mode),
                            start=(kk == 0), stop=(kk == 15),
                        )
                for n in range(2):
                    nn = half * 2 + n
                    nc.any.tensor_add(
                        ot[:, nn * 512 : (nn + 1) * 512],
                        ps[n],
                        bias_bc[:, nn * 512 : (nn + 1) * 512],
                    )
            nc.sync.dma_start(out=o_d[m * 128 : (m + 1) * 128, :], in_=ot)


def build(att_mode=ATT_MODE, mm3_mode=MM3_MODE, **emit_kwargs):
    import contextlib

    nc = bacc.Bacc("TRN2", target_bir_lowering=False, debug=False)
    q_d = nc.dram_tensor("queries", [BPC, H, L, E], F32, kind="ExternalInput").ap()
    k_d = nc.dram_tensor("keys", [BPC, H, L, E], F32, kind="ExternalInput").ap()
    v_d = nc.dram_tensor("values", [BPC, H, L, E], F32, kind="ExternalInput").ap()
    w_d = nc.dram_tensor("W", [C, C], F32, kind="ExternalInput").ap()
    b_d = nc.dram_tensor("b", [C], F32, kind="ExternalInput").ap()
    o_d = nc.dram_tensor("out", [BPC * L, C], F32, kind="ExternalOutput").ap()

    with tile.TileContext(nc) as tc:
        with contextlib.ExitStack() as ctx:
            emit(ctx, nc, tc, q_d, k_d, v_d, w_d, b_d, o_d, att_mode, mm3_mode,
                 **emit_kwargs)
    nc.compile()
    return nc


_NC_CACHE = {}


def get_nc(att_mode=ATT_MODE, mm3_mode=MM3_MODE):
    key = (att_mode, mm3_mode)
    if key not in _NC_CACHE:
        _NC_CACHE[key] = build(att_mode, mm3_mode)
    return _NC_CACHE[key]


def make_in_maps(queries, keys, values, W, b):
    queries = np.ascontiguousarray(np.asarray(queries, dtype=np.float32))
    keys = np.ascontiguousarray(np.asarray(keys, dtype=np.float32))
    values = np.ascontiguousarray(np.asarray(values, dtype=np.float32))
    W = np.ascontiguousarray(np.asarray(W, dtype=np.float32))
    b = np.ascontiguousarray(np.asarray(b, dtype=np.float32))
    in_maps = []
    for i in range(N_CORES):
        s = slice(i * BPC, (i + 1) * BPC)
        in_maps.append(
            {
                "queries": queries[s],
                "keys": keys[s],
                "values": values[s],
                "W": W,
                "b": b,
            }
        )
    return in_maps


def kernel(queries, keys, values, W, b, **run_kwargs):
    nc = get_nc()
    in_maps = make_in_maps(queries, keys, values, W, b)
    res = run_bass_kernel_spmd(nc, in_maps, core_ids=list(range(N_CORES)), **run_kwargs)
    out = np.concatenate([res.results[i]["out"] for i in range(N_CORES)], axis=0)
    return out.reshape(B, L, C)


# revision 37
# speedup vs baseline: 1.4812x; 1.1807x over previous
"""Trainium2 Bass kernel for nn_AttentionLayer (B=128,H=16,L=64,E=128, C=2048).

out[b,l,:] = (softmax(0.1 * q_bh @ k_bh^T) @ v_bh  for h) . W^T + bias

Strategy: pure data-parallel over batch across 8 NeuronCores (16 batches
per core, no collectives).  Per core:
  - attention per (batch, head-pair) group on the PE using transposed
    ("scores^T") orientation so q/k/v load in natural layout and only
    two [128,128] PE transposes per group are needed,
  - softmax without max-subtraction (scores are bounded: |0.1*s| < ~8),
    rowsum obtained by appending a ones-column to v in the U = exp@v
    matmul, normalization folded into a diag-matmul that also transposes
    U into the V^T layout the output projection needs,
  - output projection  out = V @ W^T + b  as a K=2048 accumulated matmul
    with W pre-transposed on the PE at kernel start.

Matmul dtypes are configurable (bf16 / f32r / f32) for speed vs accuracy.
"""

import numpy as np

import concourse.bass as bass
import concourse.mybir as mybir
import concourse.tile as tile
from concourse import bacc
from concourse.bass_utils import run_bass_kernel_spmd
from concourse.masks import make_identity

N_CORES = 8
B, H, L, E = 128, 16, 64, 128
C = H * E                 # 2048
BPC = B // N_CORES        # 16 batches per core
NBLK = BPC // 2           # 8 two-batch blocks per core
G = H // 2                # 8 head-pair groups per batch
SCALE = 0.1
F32 = mybir.dt.float32
BF16 = mybir.dt.bfloat16

# "bf16" | "f32r" | "f32"
ATT_MODE = "bf16"
MM3_MODE = "bf16"


def _storage_dt(mode):
    return BF16 if mode == "bf16" else F32


def _mm_ap(ap, mode):
    """View an AP with the matmul compute dtype for the given mode."""
    if mode == "f32r":
        return ap.bitcast(mybir.dt.float32r)
    return ap


def emit(ctx, nc, tc, q_d, k_d, v_d, w_d, b_d, o_d, att_mode, mm3_mode,
         skip_wprep=False, skip_attn=False, skip_mm3=False, attn_stage=99):
    att_dt = _storage_dt(att_mode)
    mm3_dt = _storage_dt(mm3_mode)

    # DRAM views: [p, b, g, e] where token row (h*64+l) = g*128 + p
    qv = q_d.rearrange("b h l e -> b (h l) e").rearrange("b (g p) e -> p b g e", p=128)
    kv = k_d.rearrange("b h l e -> b (h l) e").rearrange("b (g p) e -> p b g e", p=128)
    vv = v_d.rearrange("b h l e -> b (h l) e").rearrange("b (g p) e -> p b g e", p=128)

    # f32 weights (f32/f32r projection) eat 128KB/partition — shrink the
    # staging pools to fit
    big_w = mm3_mode != "bf16"
    # f32/f32r attention keeps q/k/v in the f32 staging tiles for the whole
    # block (no bf16 working copies) and computes the softmax row-sum with a
    # separate ones-column matmul instead of a widened v tile.
    att_f32 = att_mode != "bf16"
    const = ctx.enter_context(tc.tile_pool(name="const", bufs=1))
    wst = ctx.enter_context(tc.tile_pool(name="wst", bufs=1 if big_w else 2))
    qkvf = ctx.enter_context(
        tc.tile_pool(name="qkvf", bufs=4 if att_f32 else (2 if big_w else 4))
    )
    if not att_f32:
        qkvb = ctx.enter_context(tc.tile_pool(name="qkvb", bufs=2))
    vtp = ctx.enter_context(tc.tile_pool(name="vtp", bufs=2))
    qktp = ctx.enter_context(tc.tile_pool(name="qktp", bufs=2))
    asml = ctx.enter_context(tc.tile_pool(name="asml", bufs=2 if big_w else 4))
    outp = ctx.enter_context(tc.tile_pool(name="outp", bufs=2))

    # PSUM budget (8 banks): "tr" 3 (q/k batch transposes + V^T tiles),
    # "at" 3 (scores + U, one bank per group), + 2 banks for the projection.
    pps = ctx.enter_context(tc.tile_pool(name="pps", bufs=2, space="PSUM"))
    pat = ctx.enter_context(tc.tile_pool(name="pat", bufs=4, space="PSUM"))
    pmm3 = ctx.enter_context(tc.tile_pool(name="pmm3", bufs=2, space="PSUM"))

    # ---- constants ----
    identity_att = const.tile([128, 128], att_dt, tag="id_att")
    make_identity(nc, identity_att)
    if att_dt is F32:
        identity_f32 = identity_att
    else:
        identity_f32 = const.tile([128, 128], F32, tag="id_f32")
        make_identity(nc, identity_f32)
    if mm3_dt is att_dt:
        identity_w = identity_att
    else:
        identity_w = const.tile([128, 128], mm3_dt, tag="id_w")
        make_identity(nc, identity_w)
    # ring of pre-zeroed exp tiles: only the two diagonal 64x64 blocks are
    # ever (re)written, so the off-diagonal blocks stay zero and MM2 can
    # contract over the full 128 partitions without mixing the two heads
    EXPR = 6
    exp_ring = const.tile([128, EXPR, 128], att_dt, tag="expr")
    nc.vector.memset(exp_ring, 0.0)

    bias_bc = const.tile([128, C], F32, tag="bias")
    b_bcast = bass.AP(tensor=b_d.tensor, offset=b_d.offset, ap=[[0, 128]] + list(b_d.ap))
    nc.gpsimd.dma_start(out=bias_bc, in_=b_bcast)

    if att_f32:
        ones_col = const.tile([128, 1], F32, tag="ones")
        nc.vector.memset(ones_col, 1.0)

    # ---- first block's q/k/v loads (emit early so DMA starts early) ----
    def load_block(m):
        qf = qkvf.tile([128, 2, G, 128], F32, tag="qkvf")
        kf = qkvf.tile([128, 2, G, 128], F32, tag="qkvf")
        vf = qkvf.tile([128, 2, G, 128], F32, tag="qkvf")
        nc.sync.dma_start(out=qf, in_=qv[:, 2 * m : 2 * m + 2, :, :])
        nc.sync.dma_start(out=kf, in_=kv[:, 2 * m : 2 * m + 2, :, :])
        nc.sync.dma_start(out=vf, in_=vv[:, 2 * m : 2 * m + 2, :, :])
        if att_f32:
            return qf, kf, vf
        qb = qkvb.tile([128, 2, G, 128], att_dt, tag="qb")
        kb = qkvb.tile([128, 2, G, 128], att_dt, tag="kb")
        nc.any.tensor_copy(qb, qf)
        nc.any.tensor_copy(kb, kf)
        vb = qkvb.tile([128, 2, G, 132], att_dt, tag="vb")
        nc.any.tensor_copy(vb[:, :, :, 0:128], vf)
        nc.vector.memset(vb[:, :, :, 128:129], 1.0)
        return qb, kb, vb

    with nc.named_scope("load0"):
        blk_tiles = load_block(0)

    # ---- W preload + on-chip transpose:  wt_sb[:, kk, n] = W[n, kk*128+p] ----
    wt_sb = const.tile([128, H, C], mm3_dt, tag="wt")
    with nc.named_scope("wprep"):
        for nt in range(16) if not skip_wprep else ():
            wn_f = wst.tile([128, C], F32, tag="wnf")
            nc.sync.dma_start(out=wn_f, in_=w_d[nt * 128 : (nt + 1) * 128, :])
            if mm3_mode == "bf16":
                wn_c = wst.tile([128, C], mm3_dt, tag="wnc")
                nc.any.tensor_copy(wn_c, wn_f)
            else:
                wn_c = wn_f
            for kk in range(0, 16, 2):
                tp = pps.tile([128, 256], mm3_dt, tag="tr")
                nc.tensor.transpose(
                    tp[:, 0:128], wn_c[:, kk * 128 : (kk + 1) * 128], identity_w
                )
                nc.tensor.transpose(
                    tp[:, 128:256], wn_c[:, (kk + 1) * 128 : (kk + 2) * 128],
                    identity_w,
                )
                nc.any.tensor_copy(
                    wt_sb[:, kk : kk + 2, nt * 128 : (nt + 1) * 128],
                    tp.rearrange("p (a b) -> p a b", a=2),
                )

    # ---- output projection, emitted as a generator so its matmuls can be
    # interleaved between the NEXT block's attention groups (keeps the PE
    # dense and hot instead of alternating idle-ish attention stretches with
    # pure-projection bursts) ----
    def proj_emitter(m, vt):
        if skip_mm3:
            ot = outp.tile([128, C], F32, tag="ot_dbg")
            nc.any.tensor_copy(ot, vt.rearrange("p h t -> p (h t)"))
            nc.sync.dma_start(out=o_d[m * 128 : (m + 1) * 128, :], in_=ot)
            return
        with nc.named_scope(f"proj{m}"):
            ot = outp.tile([128, C], F32, tag="ot")
            for half in range(2):
                ps = [
                    pmm3.tile([128, 512], F32, tag="mm3", name=f"ps{n}")
                    for n in range(2)
                ]
                for kk in range(16):
                    for n in range(2):
                        nn = half * 2 + n
                        nc.tensor.matmul(
                            ps[n],
                            _mm_ap(vt[:, kk, :], mm3_mode),
                            _mm_ap(wt_sb[:, kk, nn * 512 : (nn + 1) * 512], mm3_mode),
                            start=(kk == 0), stop=(kk == 15),
                        )
                        yield
                for n in range(2):
                    nn = half * 2 + n
                    nc.any.tensor_add(
                        ot[:, nn * 512 : (nn + 1) * 512],
                        ps[n],
                        bias_bc[:, nn * 512 : (nn + 1) * 512],
                    )
                    yield
            nc.sync.dma_start(out=o_d[m * 128 : (m + 1) * 128, :], in_=ot)

    def drain(gen, k=None):
        if gen is None:
            return None
        try:
            if k is None:
                while True:
                    next(gen)
            else:
                for _ in range(k):
                    next(gen)
        except StopIteration:
            return None
        return gen

    prev_proj = None
    # ---- per-block pipeline ----
    for m in range(NBLK):
        qb, kb, vb = blk_tiles
        vt = vtp.tile([128, H, 128], mm3_dt, tag="vt")
        if skip_wprep and m == 0:
            nc.any.memset(wt_sb, 0.01)
        if skip_attn or attn_stage < 7:
            nc.any.memset(vt, 0.01)
        with nc.named_scope(f"attn{m}"):
            # batch-transpose this block's q and k up front — keeps the
            # per-group chain short and lets transposes run ahead during the
            # previous block's projection
            qkt = qktp.tile([128, 2, G, 256], att_dt, tag="qkt")
            if not skip_attn:
                for bb in range(2):
                    for g in range(G):
                        trp = pps.tile([128, 256], att_dt, tag="tr")
                        nc.tensor.transpose(trp[:, 0:128], qb[:, bb, g, :], identity_att)
                        nc.tensor.transpose(trp[:, 128:256], kb[:, bb, g, :], identity_att)
                        nc.vector.tensor_copy(qkt[:, bb, g, :], trp)
            for bb in range(2) if not skip_attn else ():
                for g in range(G):
                    prev_proj = drain(prev_proj, 5)
                    qT2 = qkt[:, bb, g, 0:128]
                    kT2 = qkt[:, bb, g, 128:256]

                    if attn_stage < 2:
                        continue
                    # One psum bank holds this group's scores^T (cols 0:128)
                    # and U' = exp@[v|1] (cols 128:257).
                    # Full 128x128 scores^T: diagonal 64x64 blocks are the two
                    # heads' k^T q; off-diagonal blocks are cross-head garbage
                    # we never read.  (Full-row matmuls only — 64-row PE tile
                    # configs fault TRN2 when consecutive matmuls move between
                    # row positions.)
                    at = pat.tile([128, 392], F32, tag="at")
                    scT = at[:, 0:128]
                    nc.tensor.matmul(
                        scT, _mm_ap(kT2, att_mode), _mm_ap(qT2, att_mode),
                        start=True, stop=True,
                    )

                    if attn_stage < 3:
                        continue
                    # exp(scale * scores^T) into a pre-zeroed ring slot: only
                    # the diagonal blocks are written, so expT is block-diagonal
                    # and MM2 can contract over all 128 partitions.
                    expT = exp_ring[:, (bb * G + g) % EXPR, :]
                    for lo, hi in ((0, 64), (64, 128)):
                        nc.scalar.activation(
                            expT[lo:hi, lo:hi], scT[lo:hi, lo:hi],
                            mybir.ActivationFunctionType.Exp, scale=SCALE,
                        )

                    if attn_stage < 4:
                        continue
                    # U = exp @ [v | 1]  -> token-major U plus rowsum column
                    U2p = at[:, 128:257]
                    if att_f32:
                        nc.tensor.matmul(
                            U2p[:, 0:128],
                            _mm_ap(expT, att_mode),
                            _mm_ap(vb[:, bb, g, 0:128], att_mode),
                            start=True, stop=True,
                        )
                        nc.tensor.matmul(
                            U2p[:, 128:129],
                            _mm_ap(expT, att_mode),
                            _mm_ap(ones_col, att_mode),
                            start=True, stop=True,
                        )
                    else:
                        nc.tensor.matmul(
                            U2p,
                            _mm_ap(expT, att_mode),
                            _mm_ap(vb[:, bb, g, 0:129], att_mode),
                            start=True, stop=True,
                        )

                    if attn_stage < 5:
                        continue
                    # normalize in token-major form: V[l2,d] = U[l2,d]/rowsum[l2]
                    # (per-partition scalar — the natural broadcast direction)
                    r2 = asml.tile([128, 1], F32, tag="r2")
                    nc.vector.reciprocal(r2, U2p[:, 128:129])
                    V2 = asml.tile([128, 128], F32, tag="V2")
                    nc.vector.tensor_scalar_mul(V2, U2p[:, 0:128], r2)

                    if attn_stage < 6:
                        continue
                    # transpose V into the c-major layout MM3's stationary needs
                    # (f32, into the spare region of this group's psum bank)
                    VT2p = at[:, 260:388]
                    nc.tensor.transpose(VT2p, V2, identity_f32)
                    if attn_stage < 7:
                        continue
                    tok = bb * 64
                    nc.vector.tensor_copy(
                        vt[:, 2 * g : 2 * g + 2, tok : tok + 64],
                        VT2p.rearrange("p (a b) -> p a b", a=2),
                    )

        # prefetch next block while this block's projection runs
        if m + 1 < NBLK:
            with nc.named_scope(f"load{m + 1}"):
                blk_tiles = load_block(m + 1)
        prev_proj = drain(prev_proj)
        prev_proj = proj_emitter(m, vt)
        if m == NBLK - 1:
            prev_proj = drain(prev_proj)



def build(att_mode=ATT_MODE, mm3_mode=MM3_MODE, **emit_kwargs):
    import contextlib

    nc = bacc.Bacc("TRN2", target_bir_lowering=False, debug=False)
    q_d = nc.dram_tensor("queries", [BPC, H, L, E], F32, kind="ExternalInput").ap()
    k_d = nc.dram_tensor("keys", [BPC, H, L, E], F32, kind="ExternalInput").ap()
    v_d = nc.dram_tensor("values", [BPC, H, L, E], F32, kind="ExternalInput").ap()
    w_d = nc.dram_tensor("W", [C, C], F32, kind="ExternalInput").ap()
    b_d = nc.dram_tensor("b", [C], F32, kind="ExternalInput").ap()
    o_d = nc.dram_tensor("out", [BPC * L, C], F32, kind="ExternalOutput").ap()

    with tile.TileContext(nc) as tc:
        with contextlib.ExitStack() as ctx:
            emit(ctx, nc, tc, q_d, k_d, v_d, w_d, b_d, o_d, att_mode, mm3_mode,
                 **emit_kwargs)
    nc.compile()
    return nc


_NC_CACHE = {}


def get_nc(att_mode=ATT_MODE, mm3_mode=MM3_MODE):
    key = (att_mode, mm3_mode)
    if key not in _NC_CACHE:
        _NC_CACHE[key] = build(att_mode, mm3_mode)
    return _NC_CACHE[key]


def make_in_maps(queries, keys, values, W, b):
    queries = np.ascontiguousarray(np.asarray(queries, dtype=np.float32))
    keys = np.ascontiguousarray(np.asarray(keys, dtype=np.float32))
    values = np.ascontiguousarray(np.asarray(values, dtype=np.float32))
    W = np.ascontiguousarray(np.asarray(W, dtype=np.float32))
    b = np.ascontiguousarray(np.asarray(b, dtype=np.float32))
    in_maps = []
    for i in range(N_CORES):
        s = slice(i * BPC, (i + 1) * BPC)
        in_maps.append(
            {
                "queries": queries[s],
                "keys": keys[s],
                "values": values[s],
                "W": W,
                "b": b,
            }
        )
    return in_maps


def kernel(queries, keys, values, W, b, **run_kwargs):
    nc = get_nc()
    in_maps = make_in_maps(queries, keys, values, W, b)
    res = run_bass_kernel_spmd(nc, in_maps, core_ids=list(range(N_CORES)), **run_kwargs)
    out = np.concatenate([res.results[i]["out"] for i in range(N_CORES)], axis=0)
    return out.reshape(B, L, C)
